# Initial kernel scaffold
#
"""Trainium2 Bass kernel for nn_MixedOp_35098472743519.

out[b, 0, :]        = 1.0                          (CLS)
out[b, p, :]        = x[b, p-1, o, :] * softmax(weights)[o]   for 1 <= p <= len_b
out[b, len_b+1, :]  = 2.0                          (SEP)
out[b, p, :]        = 0.0                          elsewhere

Sharding: pure data parallel over batch, 4 batches per core on 8 cores.
All data-dependent values (softmax weights, length masks, CLS/SEP rows) are
folded into small per-core input tensors on the host so a single SPMD program
serves every core:
  cs[p, ((b*8+k)*4+o)] = w[o] * (k*128+p < len_b)      per-partition scales
  cb[p, (b*8+k)]       = 2.0 * (k*128+p == len_b)      per-partition biases
  edge[2b+0/1, :]      = row 0 (1.0) / row 1025 (2.0 iff len_b==1024)
Device work per 128-token tile: out = (x * cs) + cb via fp32 tensor_scalar
(2x DVE mode), streamed in 2 MiB DMA chunks of 512 tokens.
"""

import os
import sys

import numpy as np

B, L, O, D = 32, 1024, 4, 256
OD = O * D            # 1024, row width in f32 elements
LP = L + 2            # 1026 output rows per batch
N_CORES = 8
BPC = B // N_CORES    # 4 batches per core
CHUNK = 512           # tokens per DMA chunk (2 MiB)
KK = CHUNK // 128     # 128-token tiles per chunk
NCHUNK = L // CHUNK   # chunks per batch

_CONCOURSE_PATHS = [
    "/opt/trn_rl_repo",
    "/root/.axon_site/_ro/trn_rl_repo",
]


def _import_concourse():
    try:
        import concourse.bass  # noqa: F401
    except ImportError:
        for p in _CONCOURSE_PATHS:
            if os.path.isdir(p) and p not in sys.path:
                sys.path.insert(0, p)
        import concourse.bass  # noqa: F401


_MODULE_CACHE = {}


def _build_module():
    if "nc" in _MODULE_CACHE:
        return _MODULE_CACHE["nc"]
    _import_concourse()
    import concourse.bass as bass
    import concourse.tile as tile
    from concourse import mybir

    f32 = mybir.dt.float32
    nc = bass.Bass("TRN2", debug=False)
    x = nc.dram_tensor("x", [BPC * L, OD], f32, kind="ExternalInput")
    cs = nc.dram_tensor("cs", [128, BPC * 8 * O], f32, kind="ExternalInput")
    cb = nc.dram_tensor("cb", [128, BPC * 8], f32, kind="ExternalInput")
    edge = nc.dram_tensor("edge", [2 * BPC, OD], f32, kind="ExternalInput")
    out = nc.dram_tensor("out", [BPC * LP, OD], f32, kind="ExternalOutput")

    x_ap = x.ap()
    out_ap = out.ap()

    with tile.TileContext(nc) as tc:
        with (
            tc.tile_pool(name="const", bufs=1) as const_pool,
            tc.tile_pool(name="xin", bufs=3) as in_pool,
            tc.tile_pool(name="yout", bufs=3) as out_pool,
        ):
            cs_t = const_pool.tile([128, BPC * 8 * O], f32)
            cb_t = const_pool.tile([128, BPC * 8], f32)
            edge_t = const_pool.tile([2 * BPC, OD], f32)
            nc.sync.dma_start(cs_t[:], cs.ap())
            nc.sync.dma_start(cb_t[:], cb.ap())
            nc.sync.dma_start(edge_t[:], edge.ap())

            # CLS row (pos 0) and final row (pos 1025) per batch.
            for b in range(BPC):
                r = b * LP
                nc.scalar.dma_start(out_ap[r : r + 1, :], edge_t[2 * b : 2 * b + 1, :])
                nc.scalar.dma_start(
                    out_ap[r + LP - 1 : r + LP, :], edge_t[2 * b + 1 : 2 * b + 2, :]
                )

            for b in range(BPC):
                for c in range(NCHUNK):
                    xr = b * L + c * CHUNK
                    src = x_ap[xr : xr + CHUNK, :].rearrange(
                        "(kk p) j -> p (kk j)", p=128
                    )
                    xt = in_pool.tile([128, KK * OD], f32)
                    nc.sync.dma_start(xt[:], src)

                    ot = out_pool.tile([128, KK * OD], f32)
                    for kk in range(KK):
                        k = c * KK + kk
                        col = b * 8 + k
                        for o in range(O):
                            lo = kk * OD + o * D
                            nc.vector.tensor_scalar(
                                ot[:, lo : lo + D],
                                xt[:, lo : lo + D],
                                cs_t[:, col * O + o : col * O + o + 1],
                                cb_t[:, col : col + 1],
                                mybir.AluOpType.mult,
                                mybir.AluOpType.add,
                            )

                    orow = b * LP + 1 + c * CHUNK
                    dst = out_ap[orow : orow + CHUNK, :].rearrange(
                        "(kk p) j -> p (kk j)", p=128
                    )
                    nc.scalar.dma_start(dst, ot[:])

    _MODULE_CACHE["nc"] = nc
    return nc


def _host_prep(x, weights, lengths):
    """Build per-core in_maps. Returns list of dicts keyed by DRAM tensor name."""
    x = np.ascontiguousarray(np.asarray(x, dtype=np.float32))
    weights = np.asarray(weights, dtype=np.float32)
    lengths = np.asarray(lengths).astype(np.int64)

    # float32 softmax, matching jax.nn.softmax(x) = exp(x - max) / sum
    m = weights.max()
    e = np.exp(weights - m, dtype=np.float32)
    w = (e / e.sum(dtype=np.float32)).astype(np.float32)

    t = np.arange(L, dtype=np.int64)
    in_maps = []
    for core in range(N_CORES):
        cs = np.empty((128, BPC * 8 * O), dtype=np.float32)
        cb = np.empty((128, BPC * 8), dtype=np.float32)
        edge = np.zeros((2 * BPC, OD), dtype=np.float32)
        for b in range(BPC):
            ln = int(lengths[core * BPC + b])
            mask = (t < ln).astype(np.float32)          # [1024]
            sep = np.where(t == ln, np.float32(2.0), np.float32(0.0))
            # mask/sep laid out [k, p] -> cs[p, (b*8+k)*O + o]
            mkp = mask.reshape(8, 128)                   # [k, p]
            skp = sep.reshape(8, 128)
            cs[:, b * 8 * O : (b + 1) * 8 * O] = (
                mkp[:, :, None] * w[None, None, :]       # [k, p, o]
            ).transpose(1, 0, 2).reshape(128, 8 * O)
            cb[:, b * 8 : (b + 1) * 8] = skp.T
            edge[2 * b, :] = 1.0
            edge[2 * b + 1, :] = 2.0 if ln == L else 0.0
        xc = x[core * BPC : (core + 1) * BPC].reshape(BPC * L, OD)
        in_maps.append({"x": xc, "cs": cs, "cb": cb, "edge": edge})
    return in_maps


def kernel(x, weights, lengths):
    _import_concourse()
    from concourse import bass_utils

    nc = _build_module()
    in_maps = _host_prep(x, weights, lengths)
    res = bass_utils.run_bass_kernel_spmd(
        nc, in_maps, core_ids=list(range(N_CORES))
    )
    out = np.stack(
        [res.results[c]["out"].reshape(BPC, LP, OD) for c in range(N_CORES)]
    ).reshape(B, LP, OD)
    return out


if __name__ == "__main__":
    xs = np.random.randn(B, L, O, D).astype(np.float32)
    ws = np.random.randn(O).astype(np.float32)
    ls = np.random.randint(1, L + 1, size=(B,)).astype(np.int64)
    y = kernel(xs, ws, ls)
    print(y.shape, y.dtype)


# revision 11
# speedup vs baseline: 9.8944x; 9.8944x over previous
"""Trainium2 Bass kernel for nn_MixedOp_35098472743519.

out[b, 0, :]        = 1.0                          (CLS)
out[b, p, :]        = x[b, p-1, o, :] * softmax(weights)[o]   for 1 <= p <= len_b
out[b, len_b+1, :]  = 2.0                          (SEP)
out[b, p, :]        = 0.0                          elsewhere

Sharding: pure data parallel over batch, 4 batches per core on 8 cores.
All data-dependent values (softmax weights, length masks, CLS/SEP rows) are
folded into small per-core input tensors on the host so a single SPMD program
serves every core:
  cs[p, ((b*8+k)*4+o)] = w[o] * (k*128+p < len_b)      per-partition scales
  cb[p, (b*8+k)]       = 2.0 * (k*128+p == len_b)      per-partition biases
  edge[2b+0/1, :]      = row 0 (1.0) / row 1025 (2.0 iff len_b==1024)
Device work per 128-token tile: out = (x * cs) + cb via fp32 tensor_scalar
(2x DVE mode), streamed in 2 MiB DMA chunks of 512 tokens.
"""

import os
import sys

import numpy as np

B, L, O, D = 32, 1024, 4, 256
OD = O * D            # 1024, row width in f32 elements
LP = L + 2            # 1026 output rows per batch
N_CORES = 8
BPC = B // N_CORES    # 4 batches per core
CHUNK = 512           # tokens per DMA chunk (2 MiB)
KK = CHUNK // 128     # 128-token tiles per chunk
NCHUNK = L // CHUNK   # chunks per batch

_CONCOURSE_PATHS = [
    "/opt/trn_rl_repo",
    "/root/.axon_site/_ro/trn_rl_repo",
]


def _import_concourse():
    try:
        import concourse.bass  # noqa: F401
    except ImportError:
        for p in _CONCOURSE_PATHS:
            if os.path.isdir(p) and p not in sys.path:
                sys.path.insert(0, p)
        import concourse.bass  # noqa: F401


_MODULE_CACHE = {}


def _build_module(reps=1):
    if ("nc", reps) in _MODULE_CACHE:
        return _MODULE_CACHE[("nc", reps)]
    _import_concourse()
    import concourse.tile as tile
    from concourse import bacc, mybir

    f32 = mybir.dt.float32
    NCS = BPC * 8 * O           # 128 scale columns
    NCB = BPC * 8               # 32 bias columns
    nc = bacc.Bacc("TRN2", debug=False, detect_race_conditions=(reps == 1))
    x = nc.dram_tensor("x", [BPC * L, OD], f32, kind="ExternalInput")
    aux = nc.dram_tensor("aux", [128, NCS + NCB], f32, kind="ExternalInput")
    edge = nc.dram_tensor("edge", [2 * BPC, OD], f32, kind="ExternalInput")
    out = nc.dram_tensor("out", [BPC * LP, OD], f32, kind="ExternalOutput")

    x_ap = x.ap()
    out_ap = out.ap()

    with tile.TileContext(nc) as tc:
        with (
            tc.tile_pool(name="const", bufs=1) as const_pool,
            tc.tile_pool(name="xin", bufs=3) as in_pool,
        ):
            aux_t = const_pool.tile([128, NCS + NCB], f32)
            edge_t = const_pool.tile([2 * BPC, OD], f32)
            nc.sync.dma_start(aux_t[:], aux.ap())
            nc.sync.dma_start(edge_t[:], edge.ap())
            cs_t = aux_t[:, :NCS]
            cb_t = aux_t[:, NCS:]

            # CLS row (pos 0) and final row (pos 1025) per batch.
            for b in range(BPC):
                r = b * LP
                nc.scalar.dma_start(out_ap[r : r + 1, :], edge_t[2 * b : 2 * b + 1, :])
                nc.scalar.dma_start(
                    out_ap[r + LP - 1 : r + LP, :], edge_t[2 * b + 1 : 2 * b + 2, :]
                )

            for b, c in [
                (b, c)
                for _ in range(reps)
                for b in range(BPC)
                for c in range(NCHUNK)
            ]:
                if True:
                    xr = b * L + c * CHUNK
                    src = x_ap[xr : xr + CHUNK, :].rearrange(
                        "(kk p) j -> p kk j", p=128
                    )
                    xt = in_pool.tile([128, KK * OD], f32)
                    nc.sync.dma_start(
                        xt[:].rearrange("p (kk j) -> p kk j", kk=KK), src
                    )

                    # in-place: out = x * cs + cb
                    for kk in range(KK):
                        k = c * KK + kk
                        col = b * 8 + k
                        for o in range(O):
                            lo = kk * OD + o * D
                            nc.vector.tensor_scalar(
                                xt[:, lo : lo + D],
                                xt[:, lo : lo + D],
                                cs_t[:, col * O + o : col * O + o + 1],
                                cb_t[:, col : col + 1],
                                mybir.AluOpType.mult,
                                mybir.AluOpType.add,
                            )

                    orow = b * LP + 1 + c * CHUNK
                    dst = out_ap[orow : orow + CHUNK, :].rearrange(
                        "(kk p) j -> p kk j", p=128
                    )
                    nc.scalar.dma_start(
                        dst, xt[:].rearrange("p (kk j) -> p kk j", kk=KK)
                    )

    nc.compile()
    _MODULE_CACHE[("nc", reps)] = nc
    return nc


def _host_prep(x, weights, lengths):
    """Build per-core in_maps. Returns list of dicts keyed by DRAM tensor name."""
    x = np.ascontiguousarray(np.asarray(x, dtype=np.float32))
    weights = np.asarray(weights, dtype=np.float32)
    lengths = np.asarray(lengths).astype(np.int64)

    # float32 softmax, matching jax.nn.softmax(x) = exp(x - max) / sum
    m = weights.max()
    e = np.exp(weights - m, dtype=np.float32)
    w = (e / e.sum(dtype=np.float32)).astype(np.float32)

    t = np.arange(L, dtype=np.int64)
    in_maps = []
    NCS = BPC * 8 * O
    for core in range(N_CORES):
        cs = np.empty((128, NCS), dtype=np.float32)
        cb = np.empty((128, BPC * 8), dtype=np.float32)
        edge = np.zeros((2 * BPC, OD), dtype=np.float32)
        for b in range(BPC):
            ln = int(lengths[core * BPC + b])
            mask = (t < ln).astype(np.float32)          # [1024]
            sep = np.where(t == ln, np.float32(2.0), np.float32(0.0))
            # mask/sep laid out [k, p] -> cs[p, (b*8+k)*O + o]
            mkp = mask.reshape(8, 128)                   # [k, p]
            skp = sep.reshape(8, 128)
            cs[:, b * 8 * O : (b + 1) * 8 * O] = (
                mkp[:, :, None] * w[None, None, :]       # [k, p, o]
            ).transpose(1, 0, 2).reshape(128, 8 * O)
            cb[:, b * 8 : (b + 1) * 8] = skp.T
            edge[2 * b, :] = 1.0
            edge[2 * b + 1, :] = 2.0 if ln == L else 0.0
        xc = x[core * BPC : (core + 1) * BPC].reshape(BPC * L, OD)
        auxc = np.concatenate([cs, cb], axis=1)
        in_maps.append({"x": xc, "aux": auxc, "edge": edge})
    return in_maps


def kernel(x, weights, lengths):
    _import_concourse()
    from concourse import bass_utils

    nc = _build_module()
    in_maps = _host_prep(x, weights, lengths)
    res = bass_utils.run_bass_kernel_spmd(
        nc, in_maps, core_ids=list(range(N_CORES))
    )
    out = np.stack(
        [res.results[c]["out"].reshape(BPC, LP, OD) for c in range(N_CORES)]
    ).reshape(B, LP, OD)
    return out


if __name__ == "__main__":
    xs = np.random.randn(B, L, O, D).astype(np.float32)
    ws = np.random.randn(O).astype(np.float32)
    ls = np.random.randint(1, L + 1, size=(B,)).astype(np.int64)
    y = kernel(xs, ws, ls)
    print(y.shape, y.dtype)
